# revision 21
# baseline (speedup 1.0000x reference)
"""DiT block kernel for 8 Trainium2 NeuronCores (Bass/Tile).

Sharding: each core owns a 256-wide query slice of the sequence (all batches,
all heads). K^T and V are all-gathered per batch (chunked collectives overlap
the attention loop). Attention bias is folded in as exp(bias) (host-
precomputed): softmax numerator = exp(s)*exp(b), applied as a DVE multiply on
the exp'd scores, which keeps the PE free of the bias preload. V carries an
embedded ones column per head so the softmax denominator rides along as psum
row 64 of the o^T accumulation.

Precision: weights (qkv/proj/w1/w3/w2/adaln) in fp8e4m3 with DoubleRow
matmuls; attention probabilities fp8e5m2; scores q,k in bf16; LN/softmax/
residual math in fp32.
"""

import contextlib
import dataclasses

import numpy as np
import ml_dtypes

import concourse.bacc as bacc
import concourse.tile as tile
from concourse import mybir
from concourse.bass_utils import run_bass_kernel_spmd

bf16 = ml_dtypes.bfloat16
f8e4 = ml_dtypes.float8_e4m3
F32 = mybir.dt.float32
BF16 = mybir.dt.bfloat16
F8 = mybir.dt.float8e4
F8E5 = mybir.dt.float8e5
AF = mybir.ActivationFunctionType
AL = mybir.AluOpType
DR = mybir.MatmulPerfMode.DoubleRow

B, N, C = 4, 2048, 768
H, D = 12, 64
E = D + 1                # v head pitch with ones column
VW = H * E               # 780
VP = 784                 # v_sb per-tile pitch (alignment)
FFN = 2048
NCORE = 8
NS = N // NCORE          # 256 queries per core
R = B * NS               # 1024 rows per core
RT = R // 128            # 8 row tiles
KT = C // 128            # 6 contraction tiles over C
KP = KT // 2             # 3 DoubleRow pairs over C
FT = FFN // 128          # 16 FFN row tiles
FP = FT // 2             # 8 DoubleRow pairs over FFN
EPS_LN = 1e-6
# m-tile groups for the attention inner loop (16 m-tiles -> 6+6+4),
# sized so each psS tile is <=3 psum banks (1536 fp32 cols).
GRP = [(0, 6), (6, 6), (12, 4)]


def _bc(ap, parts=128):
    """partition-stride-0 broadcast AP (DRAM source)."""
    return dataclasses.replace(ap, ap=[[0, parts]] + list(ap.ap))


def _bfree(ap, reps):
    """append a stride-0 free dim of size reps (SBUF broadcast operand)."""
    return dataclasses.replace(ap, ap=list(ap.ap) + [[0, reps]])


def build(collective=True, repeat=1):
    nc = bacc.Bacc("TRN2", target_bir_lowering=False, debug=False,
                   num_devices=NCORE)

    x_in = nc.dram_tensor("x", [R, C], F32, kind="ExternalInput")
    cT_in = nc.dram_tensor("cT", [C, B], F32, kind="ExternalInput")
    bias_in = nc.dram_tensor("bias_t", [H, N, NS], BF16, kind="ExternalInput")
    adw_in = nc.dram_tensor("adaln_wT", [C, 6 * C], BF16, kind="ExternalInput")
    adb_in = nc.dram_tensor("adaln_b4", [B, 6 * C], F32, kind="ExternalInput")
    qkvw_in = nc.dram_tensor("qkv_wT", [C, 3 * C], F8, kind="ExternalInput")
    qkvb_in = nc.dram_tensor("qkv_b_bc", [128, 3 * C], F32, kind="ExternalInput")
    qsc_in = nc.dram_tensor("qscale_bc", [128, C], BF16, kind="ExternalInput")
    ksc_in = nc.dram_tensor("kscale_bc", [128, C], BF16, kind="ExternalInput")
    pw_in = nc.dram_tensor("proj_wT", [C, C], F8, kind="ExternalInput")
    pb_in = nc.dram_tensor("proj_b_bc", [128, C], F32, kind="ExternalInput")
    w1_in = nc.dram_tensor("w1T", [C, FFN], BF16, kind="ExternalInput")
    w3_in = nc.dram_tensor("w3T", [C, FFN], BF16, kind="ExternalInput")
    w2_in = nc.dram_tensor("w2T", [FFN, C], BF16, kind="ExternalInput")
    w2b_in = nc.dram_tensor("w2_b_bc", [128, C], F32, kind="ExternalInput")
    id_in = nc.dram_tensor("id128", [128, 128], BF16, kind="ExternalInput")
    out_t = nc.dram_tensor("out", [R, C], F32, kind="ExternalOutput")

    with tile.TileContext(nc, num_cores=NCORE) as tc, contextlib.ExitStack() as ctx:
        consts = ctx.enter_context(tc.tile_pool(name="consts", bufs=1))
        dram = ctx.enter_context(tc.tile_pool(name="dram", bufs=1, space="DRAM"))

        eps_ln = consts.tile([128, 1], F32)
        nc.vector.memset(eps_ln, EPS_LN)
        ones_sb = consts.tile([128, 128], BF16)
        nc.vector.memset(ones_sb, 1.0)
        id_sb = consts.tile([128, 128], BF16)
        nc.sync.dma_start(out=id_sb, in_=id_in[:, :])

        for _rep in range(repeat):
            mod_dram = dram.tile([B, 6 * C], BF16)
            k_sh = [dram.tile([KT * 128, NS], BF16, name=f"k_sh{b}", tag=f"k_sh{b}")
                    for b in range(B)]
            v_sh = [dram.tile([NS, VW], F8, name=f"v_sh{b}", tag=f"v_sh{b}")
                    for b in range(B)]
            shr = "Shared" if collective else "Local"
            k_al = [dram.tile([NCORE, KT * 128 * NS], BF16, addr_space=shr,
                              name=f"k_al{b}", tag=f"k_al{b}") for b in range(B)]
            v_al = [dram.tile([NCORE, NS * VW], F8, addr_space=shr,
                              name=f"v_al{b}", tag=f"v_al{b}") for b in range(B)]

            with tc.tile_pool(name="keep", bufs=1) as keep:
                qT_sb = keep.tile([128, KT, R], BF16)     # packed q^T
                oT_sb = keep.tile([128, KT, R], F8)       # packed normalized o^T
                h2T = keep.tile([128, KT, R], BF16)       # LN2-modulated x2^T
                px_ctx = tc.tile_pool(name="px", bufs=1)
                px = px_ctx.__enter__()
                x_sb = px.tile([128, RT, C], F32)
                mv_sb = px.tile([128, RT, 2], F32)
                rstd_sb = px.tile([128, RT, 1], F32)
                p3v_ctx = tc.tile_pool(name="p3v", bufs=1)
                p3v = p3v_ctx.__enter__()
                v_sb = p3v.tile([128, 8 * NCORE, VP], F8)  # v' tiles + ones cols

                # ============ P0: x loads + LN1 stats | adaLN modulation =====
                with tc.tile_pool(name="p0", bufs=1) as p0, \
                     tc.tile_pool(name="p0c", bufs=2) as p0c, \
                     tc.tile_pool(name="p0s", bufs=3) as p0s, \
                     tc.tile_pool(name="p0ps", bufs=2, space="PSUM") as p0ps:
                    for rt in range(RT):
                        nc.gpsimd.dma_start(
                            out=x_sb[:, rt, :], in_=x_in[rt * 128:(rt + 1) * 128, :])
                        stats = p0s.tile([128, 2, 6], F32, tag="st", name="st")
                        nc.vector.bn_stats(out=stats[:, 0, :],
                                           in_=x_sb[:, rt, 0:384])
                        nc.vector.bn_stats(out=stats[:, 1, :],
                                           in_=x_sb[:, rt, 384:768])
                        nc.vector.bn_aggr(out=mv_sb[:, rt, :], in_=stats)
                    nc.scalar.activation(out=rstd_sb, in_=mv_sb[:, :, 1:2],
                                         func=AF.Sqrt, bias=eps_ln)
                    nc.vector.reciprocal(out=rstd_sb, in_=rstd_sb)

                    cT_sb = p0.tile([128, KT, B], F32)
                    nc.sync.dma_start(
                        out=cT_sb, in_=cT_in.rearrange("(t p) b -> p t b", p=128))
                    scT = p0.tile([128, KT, 16], BF16)
                    nc.vector.memset(scT, 0.0)
                    nc.scalar.activation(out=scT[:, :, 0:B], in_=cT_sb,
                                         func=AF.Silu)
                    adwg = adw_in.rearrange("(t p) j -> p t j", p=128)
                    adb_sb = p0.tile([B, 6 * C], F32)
                    nc.sync.dma_start(out=adb_sb, in_=adb_in[:, :])
                    mod_sb = p0.tile([B, 6 * C], BF16)
                    for big in range(3):
                        bsl = slice(big * 1536, (big + 1) * 1536)
                        adw_t = p0c.tile([128, KT, 1536], BF16, tag="adw_t",
                                         name="adw_t")
                        nc.scalar.dma_start(out=adw_t, in_=adwg[:, :, bsl])
                        for sub in range(3):
                            c0 = big * 1536 + sub * 512
                            sl = slice(c0, c0 + 512)
                            psM = p0ps.tile([16, 512], F32, tag="psM", name="psM")
                            for kt in range(KT):
                                nc.tensor.matmul(
                                    psM, lhsT=scT[:, kt, :],
                                    rhs=adw_t[:, kt,
                                              sub * 512:(sub + 1) * 512],
                                    start=(kt == 0), stop=(kt == KT - 1))
                            with nc.allow_low_precision(reason="bf16 mod ok"):
                                nc.vector.tensor_tensor(out=mod_sb[:, sl],
                                                        in0=psM[0:B, :],
                                                        in1=adb_sb[:, sl],
                                                        op=AL.add)
                        nc.gpsimd.dma_start(out=mod_dram[:, bsl],
                                            in_=mod_sb[:, bsl])

                # ============ P2: modulate, QKV (k,v first), rmsnorm =========
                with tc.tile_pool(name="bc1", bufs=1) as bc1, \
                     tc.tile_pool(name="p2", bufs=1) as p2, \
                     tc.tile_pool(name="p2w", bufs=3) as p2w, \
                     tc.tile_pool(name="p2ps", bufs=4, space="PSUM") as p2ps:
                    msa_sc, msa_sh = [], []
                    for b in range(B):
                        sc = bc1.tile([128, C], BF16, tag=f"sc1_{b}", name=f"sc1_{b}")
                        nc.sync.dma_start(out=sc, in_=_bc(mod_dram[b, C:2 * C]))
                        nc.vector.tensor_scalar_add(out=sc, in0=sc, scalar1=1.0)
                        sh = bc1.tile([128, C], BF16, tag=f"sh1_{b}", name=f"sh1_{b}")
                        nc.sync.dma_start(out=sh, in_=_bc(mod_dram[b, 0:C]))
                        msa_sc.append(sc)
                        msa_sh.append(sh)

                    qkvw_sb = p2.tile([128, KT, 3 * C], F8)
                    nc.scalar.dma_start(
                        out=qkvw_sb, in_=qkvw_in.rearrange("(t p) j -> p t j", p=128))
                    qkvb_sb = p2.tile([128, 3 * C], F32)
                    nc.scalar.dma_start(out=qkvb_sb, in_=qkvb_in[:, :])
                    qsc_sb = p2.tile([128, C], BF16)
                    nc.scalar.dma_start(out=qsc_sb, in_=qsc_in[:, :])
                    ksc_sb = p2.tile([128, C], BF16)
                    nc.scalar.dma_start(out=ksc_sb, in_=ksc_in[:, :])

                    h1T8 = p2.tile([128, KT, R], F8)

                    # modulate + transpose (stats already done)
                    for rt in range(RT):
                        rsl = slice(rt * 128, (rt + 1) * 128)
                        t1 = p2w.tile([128, C], BF16, tag="m_t1", name="m_t1")
                        nc.vector.tensor_scalar(
                            out=t1, in0=x_sb[:, rt, :],
                            scalar1=mv_sb[:, rt, 0:1], scalar2=rstd_sb[:, rt, :],
                            op0=AL.subtract, op1=AL.mult)
                        nc.vector.tensor_tensor(out=t1, in0=t1,
                                                in1=msa_sc[rt // 2], op=AL.mult)
                        h1b = p2w.tile([128, C], BF16, tag="m_h1", name="m_h1")
                        nc.vector.tensor_tensor(out=h1b, in0=t1,
                                                in1=msa_sh[rt // 2], op=AL.add)
                        h1Tb = p2w.tile([128, KT, 128], BF16, tag="h1Tb",
                                        name="h1Tb")
                        nc.sync.dma_start_transpose(out=h1Tb, in_=h1b)
                        with nc.allow_low_precision(reason="fp8 qkv validated"):
                            nc.gpsimd.tensor_copy(out=h1T8[:, :, rsl], in_=h1Tb)

                    def qkv_mm(rsl, c0, cw, psQ):
                        for kp in range(KP):
                            nc.tensor.matmul(
                                psQ[:, 0:cw],
                                lhsT=h1T8[:, 2 * kp:2 * kp + 2, rsl],
                                rhs=qkvw_sb[:, 2 * kp:2 * kp + 2, c0:c0 + cw],
                                start=(kp == 0), stop=(kp == KP - 1),
                                perf_mode=DR)

                    def rms_apply(t, scale_sb, ssi, dst):
                        """t [128, C] bf16 -> dst = t*scale/rms(per-head)."""
                        sq = p2w.tile([128, C], BF16, tag="sq", name="sq")
                        nc.vector.tensor_tensor(out=sq, in0=t, in1=t, op=AL.mult)
                        ssum = p2w.tile([128, 16], F32, tag="ss", name="ss")
                        nc.vector.tensor_reduce(
                            out=ssum[:, 0:H],
                            in_=sq.rearrange("p (h d) -> p h d", d=D),
                            axis=mybir.AxisListType.X, op=AL.add)
                        nc.scalar.activation(out=ssi[:, 0:H], in_=ssum[:, 0:H],
                                             func=AF.Sqrt, scale=1.0 / D)
                        nc.vector.reciprocal(out=ssi[:, 0:H], in_=ssi[:, 0:H])
                        nc.vector.tensor_tensor(out=dst, in0=t, in1=scale_sb,
                                                op=AL.mult)
                        nc.vector.tensor_tensor(
                            out=dst.rearrange("p (h d) -> p h d", d=D),
                            in0=dst.rearrange("p (h d) -> p h d", d=D),
                            in1=_bfree(ssi[:, 0:H], D), op=AL.mult)

                    # ---- k,v side; gather per batch as soon as it is ready ----
                    for rt in range(RT):
                        b, half = rt // 2, rt % 2
                        rsl = slice(rt * 128, (rt + 1) * 128)
                        kv_t = p2w.tile([128, C], BF16, tag="kv_t", name="kv_t")
                        v8_t = p2w.tile([128, VW], F8, tag="v8_t", name="v8_t")
                        v8v = v8_t.rearrange("p (h e) -> p h e", e=E)
                        nc.vector.memset(v8v[:, :, D:E], 1.0)
                        psK = p2ps.tile([128, 512], F32, tag="psQ", name="psK")
                        qkv_mm(rsl, 0, 512, psK)
                        nc.vector.tensor_tensor(out=kv_t[:, 0:512], in0=psK,
                                                in1=qkvb_sb[:, 0:512], op=AL.add)
                        psV = p2ps.tile([128, 512], F32, tag="psQ", name="psV")
                        qkv_mm(rsl, 1024, 512, psV)
                        with nc.allow_low_precision(reason="v in fp8, validated"):
                            nc.vector.tensor_tensor(
                                out=v8v[:, 4:12, 0:D],
                                in0=psV.rearrange("p (h d) -> p h d", d=D),
                                in1=qkvb_sb[:, 1024:1536].rearrange(
                                    "p (h d) -> p h d", d=D), op=AL.add)
                        # middle chunk straddles k|v
                        psM2 = p2ps.tile([128, 512], F32, tag="psQ", name="psM2")
                        qkv_mm(rsl, 512, 512, psM2)
                        nc.vector.tensor_tensor(
                            out=kv_t[:, 512:768], in0=psM2[:, 0:256],
                            in1=qkvb_sb[:, 512:768], op=AL.add)
                        with nc.allow_low_precision(reason="v in fp8, validated"):
                            nc.vector.tensor_tensor(
                                out=v8v[:, 0:4, 0:D],
                                in0=psM2[:, 256:512].rearrange(
                                    "p (h d) -> p h d", d=D),
                                in1=qkvb_sb[:, 768:1024].rearrange(
                                    "p (h d) -> p h d", d=D), op=AL.add)
                        kn_t = p2w.tile([128, C], BF16, tag="kn_t", name="kn_t")
                        ssi_k = p2w.tile([128, 16], F32, tag="ssik", name="ssik")
                        rms_apply(kv_t, ksc_sb, ssi_k, kn_t)
                        kT_t = p2w.tile([128, KT, 128], BF16, tag="kT_t",
                                        name="kT_t")
                        nc.sync.dma_start_transpose(out=kT_t, in_=kn_t)
                        nc.sync.dma_start(
                            out=k_sh[b].rearrange("(t p) n -> p t n", p=128)
                            [:, :, half * 128:(half + 1) * 128],
                            in_=kT_t)
                        nc.sync.dma_start(
                            out=v_sh[b][half * 128:(half + 1) * 128, :], in_=v8_t)
                        if half == 1:
                            if collective:
                                nc.gpsimd.collective_compute(
                                    "AllGather", AL.bypass,
                                    replica_groups=[list(range(NCORE))],
                                    ins=[k_sh[b].opt()], outs=[k_al[b].opt()])
                                nc.gpsimd.collective_compute(
                                    "AllGather", AL.bypass,
                                    replica_groups=[list(range(NCORE))],
                                    ins=[v_sh[b].opt()], outs=[v_al[b].opt()])
                            else:
                                for cc in range(2):
                                    nc.scalar.dma_start(
                                        out=k_al[b][cc:cc + 1, :],
                                        in_=k_sh[b].rearrange(
                                            "(o a) b -> o (a b)", o=1))
                                    nc.scalar.dma_start(
                                        out=v_al[b][cc:cc + 1, :],
                                        in_=v_sh[b].rearrange(
                                            "(o a) b -> o (a b)", o=1))
                            # preload v' tiles for this batch (ones embedded)
                            nc.gpsimd.dma_start(
                                out=v_sb[:, b * 16:(b + 1) * 16, 0:VW].rearrange(
                                    "p (c h) w -> p c h w", h=2),
                                in_=v_al[b].rearrange(
                                    "c (h p w) -> p c h w", h=2, p=128))

                    # ---- q side (overlaps the gathers) ----
                    for rt in range(RT):
                        rsl = slice(rt * 128, (rt + 1) * 128)
                        q_t = p2w.tile([128, C], BF16, tag="q_t", name="q_t")
                        for c0, cw, d0 in ((1536, 512, 0), (2048, 256, 512)):
                            psQ2 = p2ps.tile([128, 512], F32, tag="psQ",
                                             name="psQ2")
                            qkv_mm(rsl, c0, cw, psQ2)
                            nc.vector.tensor_tensor(
                                out=q_t[:, d0:d0 + cw], in0=psQ2[:, 0:cw],
                                in1=qkvb_sb[:, c0:c0 + cw], op=AL.add)
                        qn_t = p2w.tile([128, C], BF16, tag="qn_t", name="qn_t")
                        ssi_q = p2w.tile([128, 16], F32, tag="ssiq", name="ssiq")
                        rms_apply(q_t, qsc_sb, ssi_q, qn_t)
                        nc.sync.dma_start_transpose(out=qT_sb[:, :, rsl], in_=qn_t)

                # ============ P3: attention ============
                with tc.tile_pool(name="p3b", bufs=2) as p3b, \
                     tc.tile_pool(name="p3k", bufs=3) as p3k, \
                     tc.tile_pool(name="p3a", bufs=4) as p3a, \
                     tc.tile_pool(name="p3r", bufs=2) as p3r, \
                     tc.tile_pool(name="psS", bufs=2, space="PSUM") as psSp, \
                     tc.tile_pool(name="psO", bufs=2, space="PSUM") as psOp:
                    for g in range(KT):
                        bias_g = p3b.tile([128, 2, 16, NS], BF16, tag="bias_g",
                                          name="bias_g")
                        nc.sync.dma_start(
                            out=bias_g,
                            in_=bias_in[2 * g:2 * g + 2].rearrange(
                                "h (i p) n -> p h i n", p=128))
                        for b in range(B):
                            kT2 = p3k.tile([128, NCORE, NS], BF16, tag="kT2",
                                           name="kT2")
                            nc.sync.dma_start(
                                out=kT2,
                                in_=k_al[b].rearrange(
                                    "c (t p n) -> p c t n", t=KT, p=128)[:, :, g, :])
                            kT2f = kT2.rearrange("p c n -> p (c n)")
                            psO = [psOp.tile([128, 2 * NS], F32, tag="psO",
                                             name="psO") for _hh in range(2)]
                            for m0, mw in GRP:
                                psS = [psSp.tile([128, 1536], F32, tag="psS",
                                                 name="psS") for _hh in range(2)]
                                # bias preload via identity matmul (N=512 per
                                # bank; start=True is the bank's only
                                # non-accumulating write)
                                for t2 in range(mw // 2):
                                    for hh in range(2):
                                        nc.tensor.matmul(
                                            psS[hh][:, t2 * 512:(t2 + 1) * 512],
                                            lhsT=id_sb,
                                            rhs=bias_g[:, hh,
                                                       m0 + 2 * t2:m0 + 2 * t2 + 2,
                                                       :],
                                            start=True, stop=False,
                                            skip_group_check=True)
                                # scores: the two heads sit on row groups 0-1 /
                                # 2-3 (base partition 0 / 64), so interleaved
                                # matmuls run concurrently on the PE
                                for t in range(mw):
                                    i = m0 + t
                                    for hh in range(2):
                                        pb = hh * 64
                                        nc.tensor.matmul(
                                            psS[hh][:, t * 256:(t + 1) * 256],
                                            lhsT=kT2f[pb:pb + 64,
                                                      i * 128:(i + 1) * 128],
                                            rhs=qT_sb[pb:pb + 64, g,
                                                      b * NS:(b + 1) * NS],
                                            start=False, stop=True)
                                for hh in range(2):
                                    h = 2 * g + hh
                                    attnT = p3a.tile([128, 1536], F8E5,
                                                     tag="attnT", name="attnT")
                                    with nc.allow_low_precision(
                                            reason="fp8 attn validated"):
                                        nc.scalar.activation(
                                            out=attnT[:, 0:mw * 256],
                                            in_=psS[hh][:, 0:mw * 256],
                                            func=AF.Exp)
                                    for pr in range(mw // 2):
                                        j = m0 // 2 + pr
                                        nc.tensor.matmul(
                                            psO[hh][0:E, 0:NS],
                                            lhsT=v_sb[:, b * 16 + 2 * j:
                                                      b * 16 + 2 * j + 2,
                                                      h * E:(h + 1) * E],
                                            rhs=attnT[:, 2 * pr * 256:
                                                      (2 * pr + 2) * 256].rearrange(
                                                "p (two n) -> p two n", two=2),
                                            start=(j == 0), stop=(j == 7),
                                            perf_mode=DR)
                            for hh in range(2):
                                pb = hh * 64
                                rs = p3r.tile([128, NS], BF16, tag="rs", name="rs")
                                with nc.allow_low_precision(
                                        reason="bf16 softmax denom reciprocal"):
                                    nc.vector.reciprocal(out=rs[64:65, :],
                                                         in_=psO[hh][D:D + 1, 0:NS])
                                nc.tensor.matmul(psO[hh][:, NS:2 * NS],
                                                 lhsT=ones_sb[64:65, :],
                                                 rhs=rs[64:65, :], start=True,
                                                 stop=True, skip_group_check=True)
                                rb = p3r.tile([128, NS], BF16, tag="rb", name="rb")
                                nc.vector.tensor_copy(out=rb[0:D, :],
                                                      in_=psO[hh][0:D, NS:2 * NS])
                                with nc.allow_low_precision(
                                        reason="fp8 oT validated"):
                                    nc.vector.tensor_tensor(
                                        out=oT_sb[pb:pb + 64, g,
                                                  b * NS:(b + 1) * NS],
                                        in0=psO[hh][0:D, 0:NS], in1=rb[0:D, :],
                                        op=AL.mult)

                p3v_ctx.__exit__(None, None, None)

                # ============ P4: proj + residual + LN2 ============
                with tc.tile_pool(name="p4", bufs=1) as p4, \
                     tc.tile_pool(name="p5", bufs=1) as p5:
                    p4w_ctx = tc.tile_pool(name="p4w", bufs=3)
                    p4w = p4w_ctx.__enter__()
                    p4ps_ctx = tc.tile_pool(name="p4ps", bufs=3, space="PSUM")
                    p4ps = p4ps_ctx.__enter__()
                    bc2_ctx = tc.tile_pool(name="bc2", bufs=1)
                    bc2 = bc2_ctx.__enter__()
                    pw_sb = p4.tile([128, KT, C], F8)
                    nc.scalar.dma_start(
                        out=pw_sb, in_=pw_in.rearrange("(t p) j -> p t j", p=128))
                    pb_sb = p4.tile([128, C], F32)
                    nc.scalar.dma_start(out=pb_sb, in_=pb_in[:, :])
                    # prefetch MLP weights (fp8) during P4
                    w1_sb = p5.tile([128, KT, FFN], BF16)
                    nc.scalar.dma_start(
                        out=w1_sb, in_=w1_in.rearrange("(t p) j -> p t j", p=128))
                    w3_sb = p5.tile([128, KT, FFN], BF16)
                    nc.scalar.dma_start(
                        out=w3_sb, in_=w3_in.rearrange("(t p) j -> p t j", p=128))
                    w2_sb = p5.tile([128, FT, C], BF16)
                    nc.scalar.dma_start(
                        out=w2_sb, in_=w2_in.rearrange("(t p) j -> p t j", p=128))
                    w2b_sb = p5.tile([128, C], F32)
                    nc.scalar.dma_start(out=w2b_sb, in_=w2b_in[:, :])

                    g1_bc, mlp_sc, mlp_sh = [], [], []
                    for b in range(B):
                        g1 = bc2.tile([128, C], BF16, tag=f"g1_{b}", name=f"g1_{b}")
                        nc.sync.dma_start(out=g1, in_=_bc(mod_dram[b, 2 * C:3 * C]))
                        g1_bc.append(g1)
                        sc = bc2.tile([128, C], BF16, tag=f"sc2_{b}", name=f"sc2_{b}")
                        nc.sync.dma_start(out=sc, in_=_bc(mod_dram[b, 4 * C:5 * C]))
                        nc.vector.tensor_scalar_add(out=sc, in0=sc, scalar1=1.0)
                        sh = bc2.tile([128, C], BF16, tag=f"sh2_{b}", name=f"sh2_{b}")
                        nc.sync.dma_start(out=sh, in_=_bc(mod_dram[b, 3 * C:4 * C]))
                        mlp_sc.append(sc)
                        mlp_sh.append(sh)
                    for rt in range(RT):
                        rsl = slice(rt * 128, (rt + 1) * 128)
                        t1 = p4w.tile([128, C], F32, tag="pj_t1", name="pj_t1")
                        for c0, cw in ((0, 512), (512, 256)):
                            psP = p4ps.tile([128, 512], F32, tag="psP", name="psP")
                            for kp in range(KP):
                                nc.tensor.matmul(
                                    psP[:, 0:cw],
                                    lhsT=oT_sb[:, 2 * kp:2 * kp + 2, rsl],
                                    rhs=pw_sb[:, 2 * kp:2 * kp + 2, c0:c0 + cw],
                                    start=(kp == 0), stop=(kp == KP - 1),
                                    perf_mode=DR)
                            nc.vector.tensor_tensor(out=t1[:, c0:c0 + cw],
                                                    in0=psP[:, 0:cw],
                                                    in1=pb_sb[:, c0:c0 + cw],
                                                    op=AL.add)
                        t2 = p4w.tile([128, C], F32, tag="pj_t2", name="pj_t2")
                        nc.gpsimd.tensor_tensor(out=t2, in0=t1, in1=g1_bc[rt // 2],
                                                op=AL.mult)
                        nc.gpsimd.tensor_tensor(out=x_sb[:, rt, :],
                                                in0=x_sb[:, rt, :], in1=t2,
                                                op=AL.add)
                        # LN2 + modulate right behind each proj row tile
                        stats = p4w.tile([128, 2, 6], F32, tag="st2", name="st2")
                        nc.vector.bn_stats(out=stats[:, 0, :],
                                           in_=x_sb[:, rt, 0:384])
                        nc.vector.bn_stats(out=stats[:, 1, :],
                                           in_=x_sb[:, rt, 384:768])
                        mv2 = p4w.tile([128, 2], F32, tag="mv2", name="mv2")
                        nc.vector.bn_aggr(out=mv2, in_=stats)
                        rstd2 = p4w.tile([128, 1], F32, tag="rstd2", name="rstd2")
                        nc.scalar.activation(out=rstd2, in_=mv2[:, 1:2],
                                             func=AF.Sqrt, bias=eps_ln)
                        nc.vector.reciprocal(out=rstd2, in_=rstd2)
                        t3 = p4w.tile([128, C], F32, tag="ln2_t3", name="ln2_t3")
                        nc.vector.tensor_scalar(
                            out=t3, in0=x_sb[:, rt, :], scalar1=mv2[:, 0:1],
                            scalar2=rstd2, op0=AL.subtract, op1=AL.mult)
                        nc.vector.tensor_tensor(out=t3, in0=t3,
                                                in1=mlp_sc[rt // 2], op=AL.mult)
                        h2b = p4w.tile([128, C], BF16, tag="h2b", name="h2b")
                        nc.vector.tensor_tensor(out=h2b, in0=t3,
                                                in1=mlp_sh[rt // 2], op=AL.add)
                        nc.sync.dma_start_transpose(out=h2T[:, :, rsl], in_=h2b)

                    bc2_ctx.__exit__(None, None, None)
                    p4ps_ctx.__exit__(None, None, None)
                    p4w_ctx.__exit__(None, None, None)

                    # ============ P5: SwiGLU MLP ============
                    with tc.tile_pool(name="p5z", bufs=1) as p5z, \
                         tc.tile_pool(name="p5w", bufs=3) as p5w, \
                         tc.tile_pool(name="p5ps", bufs=2, space="PSUM") as p5ps:
                        zT_sb = p5z.tile([128, FT, R], BF16)
                        for ft in range(FT):
                            fsl = slice(ft * 128, (ft + 1) * 128)
                            for nch in range(2):
                                nsl = slice(nch * 512, (nch + 1) * 512)
                                psU = p5ps.tile([128, 512], F32, tag="psU",
                                                name="psU")
                                psG = p5ps.tile([128, 512], F32, tag="psG",
                                                name="psG")
                                for kt in range(KT):
                                    nc.tensor.matmul(
                                        psU, lhsT=w1_sb[:, kt, fsl],
                                        rhs=h2T[:, kt, nsl],
                                        start=(kt == 0), stop=(kt == KT - 1))
                                for kt in range(KT):
                                    nc.tensor.matmul(
                                        psG, lhsT=w3_sb[:, kt, fsl],
                                        rhs=h2T[:, kt, nsl],
                                        start=(kt == 0), stop=(kt == KT - 1))
                                us = p5w.tile([128, 512], BF16, tag="us", name="us")
                                nc.scalar.activation(out=us, in_=psU, func=AF.Silu)
                                gs = p5w.tile([128, 512], BF16, tag="gs", name="gs")
                                nc.scalar.copy(out=gs, in_=psG)
                                nc.vector.tensor_tensor(out=zT_sb[:, ft, nsl],
                                                        in0=us, in1=gs,
                                                        op=AL.mult)

                        # ---- z @ w2 + gate + residual ----
                        with tc.tile_pool(name="bc4", bufs=1) as bc4, \
                             tc.tile_pool(name="p6w", bufs=2) as p6w, \
                             tc.tile_pool(name="p6ps", bufs=2, space="PSUM") as p6ps:
                            g2_bc = []
                            for b in range(B):
                                g2 = bc4.tile([128, C], BF16, tag=f"g2_{b}",
                                              name=f"g2_{b}")
                                nc.sync.dma_start(
                                    out=g2, in_=_bc(mod_dram[b, 5 * C:6 * C]))
                                g2_bc.append(g2)
                            for rt in range(RT):
                                rsl = slice(rt * 128, (rt + 1) * 128)
                                t1 = p6w.tile([128, C], F32, tag="o2_t1",
                                              name="o2_t1")
                                for c0, cw in ((0, 512), (512, 256)):
                                    psP = p6ps.tile([128, 512], F32, tag="psO2",
                                                    name="psO2")
                                    for ft2 in range(FT):
                                        nc.tensor.matmul(
                                            psP[:, 0:cw],
                                            lhsT=zT_sb[:, ft2, rsl],
                                            rhs=w2_sb[:, ft2, c0:c0 + cw],
                                            start=(ft2 == 0), stop=(ft2 == FT - 1))
                                    nc.vector.tensor_tensor(
                                        out=t1[:, c0:c0 + cw], in0=psP[:, 0:cw],
                                        in1=w2b_sb[:, c0:c0 + cw], op=AL.add)
                                t2 = p6w.tile([128, C], F32, tag="o2_t2",
                                              name="o2_t2")
                                nc.gpsimd.tensor_tensor(out=t2, in0=t1,
                                                        in1=g2_bc[rt // 2],
                                                        op=AL.mult)
                                y_t = p6w.tile([128, C], F32, tag="y_t", name="y_t")
                                nc.gpsimd.tensor_tensor(out=y_t, in0=t2,
                                                        in1=x_sb[:, rt, :],
                                                        op=AL.add)
                                nc.sync.dma_start(
                                    out=out_t[rt * 128:(rt + 1) * 128, :], in_=y_t)
                px_ctx.__exit__(None, None, None)

    nc.compile()
    return nc


_CACHE = {}


def _get_nc():
    if "nc" not in _CACHE:
        _CACHE["nc"] = build()
    return _CACHE["nc"]


def prepare_in_maps(inputs):
    inputs = {k: np.asarray(v) for k, v in inputs.items()}
    x = inputs["x"].astype(np.float32)
    c = inputs["c"].astype(np.float32)
    bias = inputs["bias"].astype(np.float32)
    q_scale = inputs["q_scale"].astype(np.float32)
    k_scale = inputs["k_scale"].astype(np.float32)

    qkv_w_f = inputs["qkv_w"].astype(np.float32)          # rows [q; k; v]
    qkv_w_kvq = np.concatenate(
        [qkv_w_f[C:2 * C], qkv_w_f[2 * C:], qkv_w_f[:C]], axis=0)
    qkv_wT = np.ascontiguousarray(qkv_w_kvq.T.astype(f8e4))
    proj_wT = np.ascontiguousarray(inputs["proj_w"].astype(np.float32).T.astype(f8e4))
    w1T = np.ascontiguousarray(inputs["w1"].astype(np.float32).T.astype(bf16))
    w3T = np.ascontiguousarray(inputs["w3"].astype(np.float32).T.astype(bf16))
    w2T = np.ascontiguousarray(inputs["w2_w"].astype(np.float32).T.astype(bf16))
    adaln_wT = np.ascontiguousarray(
        inputs["adaln_w"].astype(np.float32).T.astype(bf16))
    adaln_b4 = np.ascontiguousarray(
        np.broadcast_to(inputs["adaln_b"].astype(np.float32), (B, 6 * C)))
    qkv_b_f = inputs["qkv_b"].astype(np.float32)
    qkv_b_kvq = np.concatenate([qkv_b_f[C:2 * C], qkv_b_f[2 * C:], qkv_b_f[:C]])
    qkv_b_bc = np.ascontiguousarray(np.broadcast_to(qkv_b_kvq, (128, 3 * C)))
    proj_b_bc = np.ascontiguousarray(
        np.broadcast_to(inputs["proj_b"].astype(np.float32), (128, C)))
    w2_b_bc = np.ascontiguousarray(
        np.broadcast_to(inputs["w2_b"].astype(np.float32), (128, C)))
    qscale_bc = np.ascontiguousarray(np.broadcast_to(
        np.tile(q_scale * D ** -0.5, H).astype(bf16), (128, C)))
    kscale_bc = np.ascontiguousarray(np.broadcast_to(
        np.tile(k_scale, H).astype(bf16), (128, C)))
    cT = np.ascontiguousarray(c.T)
    biasT = np.ascontiguousarray(
        bias[0].transpose(0, 2, 1).astype(bf16))   # [H, m(N), n(N)]
    id128 = np.eye(128, dtype=bf16)

    in_maps = []
    for cc in range(NCORE):
        sl = slice(cc * NS, (cc + 1) * NS)
        in_maps.append({
            "x": np.ascontiguousarray(x[:, sl, :].reshape(R, C)),
            "cT": cT,
            "bias_t": np.ascontiguousarray(biasT[:, :, sl]),
            "adaln_wT": adaln_wT, "adaln_b4": adaln_b4,
            "qkv_wT": qkv_wT, "qkv_b_bc": qkv_b_bc,
            "qscale_bc": qscale_bc, "kscale_bc": kscale_bc,
            "proj_wT": proj_wT, "proj_b_bc": proj_b_bc,
            "w1T": w1T, "w3T": w3T, "w2T": w2T, "w2_b_bc": w2_b_bc,
            "id128": id128,
        })

    return in_maps


def kernel(**inputs):
    in_maps = prepare_in_maps(inputs)
    nc = _get_nc()
    res = run_bass_kernel_spmd(nc, in_maps, core_ids=list(range(NCORE)))
    _CACHE["last_res"] = res
    out = np.empty((B, N, C), np.float32)
    for cc in range(NCORE):
        out[:, cc * NS:(cc + 1) * NS, :] = res.results[cc]["out"].reshape(B, NS, C)
    return out

